# revision 5
# baseline (speedup 1.0000x reference)
"""Bass/Trainium2 kernel for the decomposed LocallyConnected2d layer.

out[b,o,i,j] = sum_{c,k} x[b, c, i+di, j+dj] * w[o, c, i, j, k] + bias[o,i,j]
with k = di*3 + dj (3x3 kernel, stride 1).

Strategy: shard over output rows i across 8 cores (4 rows each). Each core
owns 1/8 of the per-location weight (the dominant traffic) and a 6-row halo
slice of x. Per output location (i,j) the contraction (c,k)=288 is split into
3 chunks of 96 = (c,di) indexed, chunked over dj; each chunk is one matmul
lhsT=[96,64] rhs=[96,128] accumulating into PSUM [64 o, 128 b]. The bias is
folded into the dj=2 chunk as a 97th contraction row against a constant-ones
rhs partition; the ones ride along as a 33rd all-ones x channel so they cost
no extra DMA. Even/odd j use PE column groups 0/1 (tile_position) so two
locations' matmuls overlap in the array. All matmul data is fp16 (PE runs
fp16 at 4x the fp32 rate; fp32 accumulate in PSUM); output is written fp16
and upcast on the host.

The kernel is DMA-bound (~10.3 MB across w/x/out vs ~300 GB/s of per-core
DMA throughput, vs only ~11 us of PE work), so the schedule keeps the 16
DMA engines saturated from the first possible cycle and minimizes the work
gated by the *last-arriving* transfer:
  - Sync's HWDGE ring carries only weights, split per output row (wa_r,
    wb_r, wc_r; the final row's wc in two half-row pieces) in exactly the
    order the in-order PE consumes them: the first matmul triple needs only
    ~1.2 MB of weights, and the last 264 KB piece gates just 48 matmuls +
    one copy + one 264 KB output transfer.
  - Scalar's ring carries x (one slab per output row, so row r's matmuls
    never wait on later rows' x) followed by the 8 half-row output DMAs;
    the two rings carry ~5 MB each so both drain together.
  - The bias row rides inside each wc_r DMA (host weight rows 192..288 are
    contiguous).
  - Each PSUM accumulation group stays a consecutive wa,wb,wc triple (the
    scheduler mis-tracks interleaved start/stop groups), PSUM->SBUF copies
    alternate Vector/Scalar per group.
"""

import sys

for _p in ("/opt/trn_rl_repo", "/root/.axon_site/_ro/trn_rl_repo"):
    if _p not in sys.path:
        sys.path.append(_p)

import numpy as np

B = 128
C_IN = 32
C_OUT = 64
OH = OW = 32
KH = KW = 3
H = W = 34
N_CORES = 8
RPC = OH // N_CORES          # output rows per core = 4
HALO = RPC + KH - 1          # x rows per core = 6
NPAIR = OW // 2              # j-pairs per row = 16
NGRP = 4                     # j-pairs per psum group
GRPS = NPAIR // NGRP         # psum groups per row = 4

_DT_MM = "float16"           # matmul operand dtype
_DT_OUT = "float16"          # device output dtype

_prog_cache = {}


def _build_program():
    import concourse.tile as tile
    from concourse import bacc, mybir
    from bass_rust import AP

    dt_mm = getattr(mybir.dt, _DT_MM)
    dt_out = getattr(mybir.dt, _DT_OUT)
    f32 = mybir.dt.float32

    nc = bacc.Bacc("TRN2", target_bir_lowering=False, debug=False,
                   num_devices=N_CORES)

    # Per-core DRAM I/O (host pre-sharded / pre-transposed):
    #   x_in  [c=33, h=6, w=34, b=128]  halo slice, b innermost; c=32 is 1.0
    #   w_in  [r=289, i=4, j=32, o=64]  r = dj*96 + c*3 + di; r=288 is bias
    #   out   [p2=128 (par*64+o), i=4, jh=16, b=128] ; j = 2*jh + par
    x_in = nc.dram_tensor("x", [C_IN + 1, HALO, W, B], dt_mm,
                          kind="ExternalInput").ap()
    w_in = nc.dram_tensor("w", [289, RPC, OW, C_OUT], dt_mm,
                          kind="ExternalInput").ap()
    out = nc.dram_tensor("out", [128, RPC, NPAIR, B], dt_out,
                         kind="ExternalOutput").ap()

    HSTR = W * B                # x_in h-row stride (elements)
    CSTR = HALO * W * B         # x_in c stride

    with tile.TileContext(nc) as tc:
        with (
            tc.tile_pool(name="xpool", bufs=1) as xpool,
            tc.tile_pool(name="wpool", bufs=1) as wpool,
            tc.tile_pool(name="opool", bufs=4) as opool,
            tc.tile_pool(name="pspool", bufs=8, space="PSUM") as pspool,
        ):
            # x slabs, one per output row: partition p = c*3+di holds image
            # row r+di of channel c; partition 96 is the all-ones channel
            # (97/98 are ones too, unused). 8.7KB runs, c outermost so the
            # HWDGE spreads packets across all 16 engines.
            xs = [xpool.tile([99, W, B], dt_mm, tag=f"xs{r}",
                             name=f"xs{r}")
                  for r in range(RPC)]
            for r in range(RPC):
                src = AP(x_in.tensor, r * HSTR,
                         [(CSTR, C_IN + 1), (HSTR, KH), (1, W * B)])
                nc.scalar.dma_start(xs[r][:], src)

            # Weights: per row, three contraction-chunk tiles; wc_r holds the
            # bias as partition 96 (host rows 192..288 contiguous -> one
            # transfer each, 4KB runs). The final row's wc arrives in two
            # half-row pieces so the very last transfer gates only two PSUM
            # groups. Sync's ring sees them in PE consumption order.
            wa = [wpool.tile([96, OW, C_OUT], dt_mm, tag=f"wa{r}",
                             name=f"wa{r}")
                  for r in range(RPC)]
            wb = [wpool.tile([96, OW, C_OUT], dt_mm, tag=f"wb{r}",
                             name=f"wb{r}")
                  for r in range(RPC)]
            wc = [wpool.tile([97, OW, C_OUT], dt_mm, tag=f"wc{r}",
                             name=f"wc{r}")
                  for r in range(RPC)]
            for r in range(RPC):
                nc.sync.dma_start(wa[r][:], w_in[0:96, r])
                nc.sync.dma_start(wb[r][:], w_in[96:192, r])
                if r < RPC - 1:
                    nc.sync.dma_start(wc[r][:], w_in[192:289, r])
                else:
                    half = OW // 2
                    nc.sync.dma_start(wc[r][:, 0:half, :],
                                      w_in[192:289, r, 0:half, :])
                    nc.sync.dma_start(wc[r][:, half:OW, :],
                                      w_in[192:289, r, half:OW, :])

            for i in range(RPC):
                for hh in range(2):
                    oh = opool.tile([128, 2 * NGRP, B], dt_out, tag="op")
                    for gg in range(2):
                        g = 2 * hh + gg
                        ps = pspool.tile([128, NGRP, B], f32)
                        for pig in range(NGRP):
                            for par in range(2):
                                j = 2 * (NGRP * g + pig) + par
                                pslice = ps[64 * par:64 * par + 64, pig, :]
                                tp = (0, 64 * par)
                                nc.tensor.matmul(pslice, wa[i][:, j, :],
                                                 xs[i][0:96, j, :],
                                                 start=True, stop=False,
                                                 tile_position=tp)
                                nc.tensor.matmul(pslice, wb[i][:, j, :],
                                                 xs[i][0:96, j + 1, :],
                                                 start=False, stop=False,
                                                 tile_position=tp)
                                nc.tensor.matmul(pslice, wc[i][:, j, :],
                                                 xs[i][0:97, j + 2, :],
                                                 start=False, stop=True,
                                                 tile_position=tp)
                        dst = oh[:, NGRP * gg:NGRP * (gg + 1), :]
                        if g % 2 == 0:
                            nc.vector.tensor_copy(dst, ps[:])
                        else:
                            nc.scalar.copy(dst, ps[:])
                    nc.scalar.dma_start(
                        out[:, i, NGRP * 2 * hh:NGRP * 2 * (hh + 1), :],
                        oh[:])

    nc.compile()
    return nc


def _host_prep(x, weight, bias):
    """Full fp32 inputs -> list of per-core input dicts."""
    np_mm = np.dtype(_DT_MM)
    # x: (B, C, H, W) -> (C+1, H, W, B) with an all-ones channel appended
    x_t = np.ascontiguousarray(x.transpose(1, 2, 3, 0)).astype(np_mm)
    x_t = np.concatenate([x_t, np.ones((1, H, W, B), np_mm)], axis=0)
    # w: (O, C, I, J, K) -> [(dj,c,di)=288, i, j, o], bias appended as row 288
    w_r = weight.reshape(C_OUT, C_IN, OH, OW, KH, KW)
    w_t = w_r.transpose(5, 1, 4, 2, 3, 0).reshape(288, OH, OW, C_OUT)
    b_t = bias.transpose(1, 2, 0)[None]                   # (1, I, J, O)
    w_aug = np.concatenate([w_t, b_t], axis=0).astype(np_mm)  # (289, I, J, O)

    in_maps = []
    for m in range(N_CORES):
        r0 = m * RPC
        in_maps.append({
            "x": np.ascontiguousarray(x_t[:, r0:r0 + HALO]),
            "w": np.ascontiguousarray(w_aug[:, r0:r0 + RPC]),
        })
    return in_maps


def _gather(results):
    out_full = np.empty((B, C_OUT, OH, OW), np.float32)
    for m in range(N_CORES):
        r = results[m]["out"].astype(np.float32)          # (128, 4, 16, 128)
        r = r.reshape(2, C_OUT, RPC, NPAIR, B)            # par,o,i,jh,b
        r = r.transpose(4, 1, 2, 3, 0)                    # b,o,i,jh,par
        out_full[:, :, m * RPC:(m + 1) * RPC, :] = r.reshape(B, C_OUT, RPC, OW)
    return out_full


def kernel(x, weight, bias, _trace=False):
    from concourse.bass_utils import run_bass_kernel_spmd

    if "nc" not in _prog_cache:
        _prog_cache["nc"] = _build_program()
    nc = _prog_cache["nc"]

    in_maps = _host_prep(np.asarray(x), np.asarray(weight), np.asarray(bias))
    res = run_bass_kernel_spmd(nc, in_maps, core_ids=list(range(N_CORES)),
                               trace=_trace)
    out = _gather(res.results)
    if _trace:
        _prog_cache["last_result"] = res
    return out
